# revision 27
# baseline (speedup 1.0000x reference)
"""CoPEGate Trainium2 kernel.

Computes out[b,h,t,s] = sigmoid((Q K^T)[b,h,t,s] / sqrt(D)) * (P P^T)[t,s] / sqrt(D)
for B=2, H=12, T=2048, D=64 (fp32 in/out), distributed over 8 NeuronCores.

Sharding: the 24 (b,h) pairs are split 3-per-core (head-parallel); the
positional matrix P is replicated and its T x T bias is computed on every
core (reused across that core's 3 heads). No cross-device communication.

Key TRN2 microarch facts driving the design (measured on hw):
  * PE matmuls with K<=64 contraction stream moving columns at HALF rate
    (427ns per 512 fp16 cols) while K=128 streams at full rate (216ns).
    Since D=64, each 128-row output stripe is computed as ONE K=128
    block-diagonal matmul: lhsT = blockdiag(qT[:, t0:t0+64], qT[:,
    t0+64:t0+128]) (host-precomputed, fp16) against rhs = [k; k]
    (partition-duplicated, fp16).  Output partition p = t-row t0+p,
    exactly as a K=64 pair would give, at 2x the PE rate.
  * Output is written as fp16 (rel-err ~4e-4 << 2e-2 budget), halving
    HBM write traffic to 24 MiB/core.
  * Scalar engine (153.6 Ge/s) does the 3 sigmoid stripes per tile;
    vector (DVE) does the pos PSUM->SBUF fp16 copy (1x) and the
    gate*pos fp16 multiplies (2x mode).

Per-core dataflow: 16 row-tiles x 4 stripes (pos, h0, h1, h2) of
[128, 2048]: PE matmul -> PSUM; scalar Sigmoid -> fp16 SBUF (heads) /
vector scaled-copy -> fp16 SBUF (pos); vector multiply -> fp16 SBUF;
one 512 KiB contiguous DMA per head-stripe.
"""

import math
import os
import sys

import numpy as np

sys.path.insert(0, "/opt/trn_rl_repo")

B, H, T, D = 2, 12, 2048, 64
N_CORES = 8
HPC = (B * H) // N_CORES  # heads per core
PT = 128  # output row-tile height (SBUF/PSUM partitions)
NT = T // PT  # row tiles
NCHUNK = 512  # matmul moving-operand free dim (one PSUM bank of fp32)
NCH = T // NCHUNK
INV_SQRT_D = 1.0 / math.sqrt(D)

# Clamped-cubic sigmoid approximation for the custom DVE op:
# sigmoid(x) ~= 0.5 + u*(A + B*u^2), u = clip(x, -3, 3); N(0,1)-weighted
# rms err 4.7e-3 (used on ~25% of gate elements -> ~4.3e-3 global rel err,
# well under the 2e-2 budget).
A_COEF = 0.23985824898098862
B_COEF = -0.011201461086009844

# Offload ~1/4 of the sigmoid stripes to the vector engine via the custom
# DVE op (False falls back to all-scalar sigmoids).
USE_APPROX = True

_NC_CACHE = {}


def _register_sigmoid_op():
    """Register (once) a custom micro-coded DVE op computing the clamped
    cubic sigmoid approximation, so the vector engine can take sigmoid work
    off the scalar engine. 7 of 8 ALU stages; the +-3 clamp bounds are
    hoisted from hardware One constants so all three scalar slots remain
    for a, b, 0.5."""
    import dataclasses

    from concourse import dve_ops
    from concourse.dve_spec import (
        C0,
        C1,
        C2,
        Spec,
        Src0,
        Src1,
        Zero,
        _has_src1,
        lower,
        maxx,
        minn,
        sq,
    )
    from concourse.dve_uop import DveOpSpec

    for o in dve_ops.OPS:
        if o.name == "SIGMOID_CUBE_ANT":
            return o

    u = maxx(minn(Src0, C0), Zero - C0)
    body = u * (sq(u) * C1 + Src1) + C2

    def _ref(in0, in1, s0, s1, imm2):
        uu = np.clip(in0.astype(np.float32), -s0, s0)
        return (uu * (uu * uu * s1 + in1) + imm2).astype(np.float32)

    spec = Spec(body=body, reference=_ref)
    tmp = dve_ops.DveOp("SIGMOID_CUBE_ANT", spec, subdim=False, uops_sha={})
    dve_ops.OPS.append(tmp)
    # OPS-derived registries are materialized at dve_ops import; extend them
    # for the appended op (row = base + index, same rule as the comprehension).
    dve_ops.CUSTOM_DVE_SPECS[tmp.name] = spec
    dve_ops._SUB_OPCODE_FOR_NAME[tmp.name] = (
        dve_ops._CUSTOM_DVE_ROW_BASE + len(dve_ops.OPS) - 1
    )
    ds = DveOpSpec(
        name="SIGMOID_CUBE_ANT",
        opcode=dve_ops.get_dve_sub_opcode("SIGMOID_CUBE_ANT"),
        uops=lower(spec, ver="v3"),
        rd1_en=_has_src1(spec),
    )
    op = dataclasses.replace(tmp, uops_sha={"v3": ds.sha("v3")})
    dve_ops.OPS[dve_ops.OPS.index(tmp)] = op
    return op


def _build_nc():
    import concourse.bass as bass
    from concourse import bacc, mybir, tile

    f32 = mybir.dt.float32
    f16 = mybir.dt.float16
    Sigmoid = mybir.ActivationFunctionType.Sigmoid
    Copy = mybir.ActivationFunctionType.Copy

    sig_op = _register_sigmoid_op()

    nc = bacc.Bacc("TRN2", target_bir_lowering=False)

    # qbd: per head, 16 block-diagonal [128(K), 128(M)] weight tiles packed
    # along the free dim -> [128, 2048]. kd: k duplicated in both partition
    # halves -> [128(K), 2048(N)]. pbd/pd: same for the positional matrix
    # (host pre-scaled by 1/sqrt(sqrt(D)) so pos PSUM = P P^T / sqrt(D)).
    qbd = nc.dram_tensor("qbd", [HPC, PT, NT * PT], f16, kind="ExternalInput")
    kd = nc.dram_tensor("kd", [HPC, PT, T], f16, kind="ExternalInput")
    pbd = nc.dram_tensor("pbd", [PT, NT * PT], f16, kind="ExternalInput")
    pd = nc.dram_tensor("pd", [PT, T], f16, kind="ExternalInput")
    out = nc.dram_tensor("out", [HPC, T, T], f16, kind="ExternalOutput")

    with tile.TileContext(nc) as tc:
        with tc.tile_pool(name="ins", bufs=1) as ins_pool, \
             tc.tile_pool(name="pos", bufs=2) as pos_pool, \
             tc.tile_pool(name="gate", bufs=6) as gate_pool, \
             tc.tile_pool(name="outs", bufs=12) as outs_pool, \
             tc.tile_pool(name="ps", bufs=2, space="PSUM") as ps_pool:

            # Inputs live as 512-wide column-chunk tiles so the first
            # matmul only waits for a fraction of the input DMA
            # (separate tiles => separate scheduler dependencies).
            k_c = [[None] * NCH for _ in range(HPC)]
            q_c = [[None] * NCH for _ in range(HPC)]
            p_c = [None] * NCH
            pq_c = [None] * NCH
            jsl = lambda j: bass.ts(j, NCHUNK)

            def load(lst, j, name, src):
                t = ins_pool.tile([PT, NCHUNK], f16, tag=f"{name}_{j}")
                nc.sync.dma_start(out=t, in_=src[:, jsl(j)])
                lst[j] = t

            # DMA issue order follows tile-0's dataflow so the pipeline
            # ramps as early as possible (each SP dma_start costs ~700ns of
            # sequencer time, so issue order = output start latency).
            # Only tile-0's operands are loaded up front; h1/h2 operands are
            # issued right before their stripes, and the weight chunks for
            # tiles 4+ are drip-fed from `deferred` between later stripes.
            # Full-size tile carrying the cubic's linear coefficient via the
            # Src1 stream (the op's 3 scalar slots hold the clamp bound, b,
            # and 0.5; a [P,1] broadcast in1 faults on hw, so it's full-size).
            acoef = ins_pool.tile([PT, T], f32, tag="acoef")
            nc.vector.memset(acoef, A_COEF)

            load(k_c[0], 0, "k0", kd[0])
            load(q_c[0], 0, "q0", qbd[0])
            load(p_c, 0, "p", pd)
            load(pq_c, 0, "pq", pbd)
            for j in range(1, NCH):
                load(k_c[0], j, "k0", kd[0])
                load(p_c, j, "p", pd)

            deferred = []
            for j in range(1, NCH):
                deferred.append((pq_c, j, "pq", pbd))
                for h in range(HPC):
                    deferred.append((q_c[h], j, f"q{h}", qbd[h]))

            def drip():
                if deferred:
                    load(*deferred.pop(0))

            def lhsT(h, it):
                # [128, 128] block-diagonal weight tile for row-tile `it`.
                sl = bass.ts(it % (NCHUNK // PT), PT)
                src = pq_c if h is None else q_c[h]
                return src[it // (NCHUNK // PT)][:, sl]

            def rhs(h, j):
                return p_c[j][:, :] if h is None else k_c[h][j][:, :]

            def mm_stripe(psum, h, it):
                w = lhsT(h, it)
                for j in range(NCH):
                    nc.tensor.matmul(
                        psum[:, jsl(j)], w, rhs(h, j), start=True, stop=True
                    )

            def post(h, sp, pos_sb, tsl, mul_eng=None, approx=False):
                gate = gate_pool.tile([PT, T], f16, tag="gate")
                o = outs_pool.tile([PT, T], f16, tag="o")
                if approx:
                    # vector-engine clamped-cubic sigmoid (scores arrive
                    # pre-scaled by 1/sqrt(D) via the host q scaling)
                    nc.vector._custom_dve(
                        sig_op, out=gate, in0=sp, in1=acoef[:, :], s0=3.0,
                        s1=B_COEF, imm2=0.5,
                    )
                else:
                    nc.scalar.activation(gate, sp, Sigmoid, scale=1.0)
                (mul_eng or nc.vector).tensor_mul(o, gate, pos_sb)
                nc.sync.dma_start(out=out[h, tsl, :], in_=o)

            # --- tile 0: chunk-major software pipeline -----------------
            # Emitted in dataflow order, chunk by chunk, so every engine's
            # FIFO sees tile 0's chunk c before chunk c+1 work and the
            # first output bytes reach HBM as early as possible.
            tsl0 = bass.ts(0, PT)
            sp0 = ps_pool.tile([PT, T], f32, tag="ps")
            pp0 = ps_pool.tile([PT, T], f32, tag="ps")
            pos0 = pos_pool.tile([PT, T], f16, tag="pos")
            gate0 = gate_pool.tile([PT, T], f16, tag="gate")
            o0 = outs_pool.tile([PT, T], f16, tag="o")
            for c in range(NCH):
                csl = jsl(c)
                nc.tensor.matmul(
                    sp0[:, csl], lhsT(0, 0), rhs(0, c), start=True, stop=True
                )
                nc.tensor.matmul(
                    pp0[:, csl], lhsT(None, 0), rhs(None, c), start=True,
                    stop=True,
                )
                nc.scalar.activation(
                    gate0[:, csl], sp0[:, csl], Sigmoid, scale=1.0
                )
                nc.vector.tensor_scalar_mul(pos0[:, csl], pp0[:, csl], 1.0)
                nc.vector.tensor_mul(o0[:, csl], gate0[:, csl], pos0[:, csl])
                nc.sync.dma_start(out=out[0, tsl0, csl], in_=o0[:, csl])
            for h in (1, 2):
                load(k_c[h], 0, f"k{h}", kd[h])
                load(q_c[h], 0, f"q{h}", qbd[h])
                for j in range(1, NCH):
                    load(k_c[h], j, f"k{h}", kd[h])
                sp = ps_pool.tile([PT, T], f32, tag="ps")
                mm_stripe(sp, h, 0)
                post(h, sp, pos0, tsl0)

            # --- steady-state tiles ------------------------------------
            # Engine balance per tile: scalar = 2 sigmoids + 1/4 pos copy,
            # vector = 3/4 pos copy + 2 muls (2x fp16 mode), gpsimd = 1 mul
            # + 1 sigmoid's worth... no: gpsimd = 1 mul (0.42-efficiency Q7),
            # PE ~52% on matmuls, so no engine exceeds ~5.3us/tile.
            CSPL = 512  # pos-copy columns handled by scalar
            for it in range(1, NT):
                tsl = bass.ts(it, PT)
                pp = ps_pool.tile([PT, T], f32, tag="ps")
                mm_stripe(pp, None, it)
                pos_sb = pos_pool.tile([PT, T], f16, tag="pos")
                nc.vector.tensor_scalar_mul(
                    pos_sb[:, CSPL:], pp[:, CSPL:], 1.0
                )
                nc.scalar.activation(
                    pos_sb[:, :CSPL], pp[:, :CSPL], Copy, scale=1.0
                )
                drip()
                for h in range(HPC):
                    sp = ps_pool.tile([PT, T], f32, tag="ps")
                    mm_stripe(sp, h, it)
                    post(h, sp, pos_sb, tsl,
                         mul_eng=nc.gpsimd if h == 1 else None,
                         approx=(USE_APPROX and h == 2 and it % 4 != 0))
                    drip()

    nc.finalize()
    return nc


def _get_nc():
    if "nc" not in _NC_CACHE:
        _NC_CACHE["nc"] = _build_nc()
    return _NC_CACHE["nc"]


def _host_pack(query, key, pos_embed_weight):
    """Build per-core input maps: block-diag fp16 weight tiles + duplicated
    moving operands."""
    # q pre-scaled by 1/sqrt(D): scores leave the PE already scaled, so both
    # the scalar Sigmoid (scale=1.0) and the custom DVE cubic see x/sqrt(D).
    q = (query.reshape(B * H, T, D) * INV_SQRT_D).astype(np.float16)
    k = key.reshape(B * H, T, D).astype(np.float16)
    p = (pos_embed_weight[:T] * (INV_SQRT_D ** 0.5)).astype(np.float16)

    # kd[h]: [128, T] = [k^T; k^T] (K-duplicated moving operand)
    kT = np.ascontiguousarray(k.transpose(0, 2, 1))  # [BH, 64, T]
    kdup = np.concatenate([kT, kT], axis=1)  # [BH, 128, T]
    pT = np.ascontiguousarray(p.T)  # [64, T]
    pdup = np.concatenate([pT, pT], axis=0)  # [128, T]

    # Block-diagonal weight tiles, packed along free dim:
    # qbd[h][:, it*128:(it+1)*128] = blockdiag(q[h, it*128:+64].T,
    #                                          q[h, +64:+128].T)
    def blockdiag_tiles(x):  # x: [N, T, 64] -> [N, 128, NT*128]
        n = x.shape[0]
        bd = np.zeros((n, NT, PT, PT), dtype=np.float16)
        xr = x.reshape(n, NT, 2, 64, D)  # [N, NT, half, 64(t), 64(d)]
        bd[:, :, :D, :D] = xr[:, :, 0].transpose(0, 1, 3, 2)
        bd[:, :, D:, D:] = xr[:, :, 1].transpose(0, 1, 3, 2)
        return np.ascontiguousarray(
            bd.transpose(0, 2, 1, 3).reshape(n, PT, NT * PT)
        )

    qbd = blockdiag_tiles(q)
    pbd = blockdiag_tiles(p[None])[0]

    in_maps = []
    for c in range(N_CORES):
        hs = slice(c * HPC, (c + 1) * HPC)
        in_maps.append(
            {
                "qbd": np.ascontiguousarray(qbd[hs]),
                "kd": np.ascontiguousarray(kdup[hs]),
                "pbd": pbd,
                "pd": pdup,
            }
        )
    return in_maps


def kernel(query, key, pos_embed_weight):
    query = np.asarray(query, dtype=np.float32)
    key = np.asarray(key, dtype=np.float32)
    pos_embed_weight = np.asarray(pos_embed_weight, dtype=np.float32)

    in_maps = _host_pack(query, key, pos_embed_weight)

    from concourse.bass_utils import run_bass_kernel_spmd

    nc = _get_nc()
    try:
        res = run_bass_kernel_spmd(
            nc,
            in_maps,
            core_ids=list(range(N_CORES)),
            trace=bool(os.environ.get("KERNEL_TRACE")),
        )
    except Exception:
        # One retry for transient runtime/compile hiccups.
        res = run_bass_kernel_spmd(
            nc, in_maps, core_ids=list(range(N_CORES)), trace=False
        )
    kernel.last_results = res

    full = np.empty((B * H, T, T), dtype=np.float32)
    for c in range(N_CORES):
        full[c * HPC : (c + 1) * HPC] = res.results[c]["out"]  # f16 -> f32
    return full.reshape(B, H, T, T)


kernel.last_results = None
